# revision 2
# baseline (speedup 1.0000x reference)
"""Two-layer GCN on 8 Trainium2 NeuronCores via Bass/Tile.

Strategy (dst-sharded, per sharding hint):
- Nodes split into 8 dst-ranges of 12500 (one per core); each core aggregates
  messages for its own dst range.
- GCN algebra: with h' = (x@W1)*dis (dis = deg^-1/2), layer output =
  relu(dis*(S + 0) + b1) where S[d] = h'[d] + sum_{e: dst=d} h'[src].
  Same for layer 2 with a1' = relu-out scaled by dis, head (S2*dis)@W2+b2.
- Aggregation: dma_gather (GPSIMD SWDGE, 4 dynamic queues) from an f32
  [*, 64] table (256B rows) using int16 slot tables; the int16 limit forces
  4 table chunks of 25001 rows (local row 25000 is a zero row for padding).
  Each chunk-pass uses its own degree-sorted dst permutation to keep slot
  padding ~5%; 4 permuted partial accumulators are then combined by a small
  second gather.
"""
import numpy as np

N = 100000
E_CH = 128
HID = 64
OUT = 2
NC = 8
PERCORE = 12500
PC_PAD = 12544           # 98 tiles of 128
NTILES = PC_PAD // 128
CHUNK = 25000
CHUNK_ROWS = CHUNK + 1   # + zero row
NCHUNK = 4
MAX_CALL_COLS = 96       # staging cap per dma_gather call


def _wrap_idx(vals):
    """positions i -> (partition i%16, col i//16), replicated to 128 partitions."""
    ni = len(vals)
    assert ni % 16 == 0
    return np.tile(vals.reshape(ni // 16, 16).T, (8, 1))


def _host_prep(edge_index):
    src = np.asarray(edge_index[0], dtype=np.int64)
    dst = np.asarray(edge_index[1], dtype=np.int64)
    deg = np.bincount(dst, minlength=N).astype(np.float64) + 1.0
    dis = (1.0 / np.sqrt(deg)).astype(np.float32)

    # add self loops as ordinary edges
    allsrc = np.concatenate([src, np.arange(N, dtype=np.int64)])
    alldst = np.concatenate([dst, np.arange(N, dtype=np.int64)])
    chunk = allsrc // CHUNK
    core = alldst // PERCORE
    dloc = alldst % PERCORE
    sloc = (allsrc % CHUNK).astype(np.int64)

    # per (core, chunk): per-dst counts
    cnt = np.bincount((alldst * NCHUNK + chunk), minlength=N * NCHUNK).reshape(N, NCHUNK)

    perms = np.zeros((NC, NCHUNK, PC_PAD), np.int64)       # tile order -> dst_local
    invperms = np.zeros((NC, NCHUNK, PC_PAD), np.int64)    # dst_local -> row in perm
    K = np.zeros((NCHUNK, NTILES), np.int64)               # cross-core max slots per tile
    for i in range(NC):
        c0 = cnt[i * PERCORE:(i + 1) * PERCORE]            # [12500, 4]
        for c in range(NCHUNK):
            p = np.argsort(-c0[:, c], kind="stable")
            p = np.concatenate([p, np.arange(PERCORE, PC_PAD)])
            perms[i, c] = p
            inv = np.empty(PC_PAD, np.int64)
            inv[p] = np.arange(PC_PAD)
            invperms[i, c] = inv
            cc = np.concatenate([c0[:, c], np.zeros(PC_PAD - PERCORE, np.int64)])
            kt = cc[p].reshape(NTILES, 128).max(axis=1)
            K[c] = np.maximum(K[c], kt)
    K = np.maximum(K, 1)

    # slot matrices M[core][chunk]: [PC_PAD, K_c_max] int16 local src (pad=CHUNK)
    Ms = [[None] * NCHUNK for _ in range(NC)]
    for i in range(NC):
        esel_core = core == i
        for c in range(NCHUNK):
            sel = esel_core & (chunk == c)
            ed = dloc[sel]
            es = sloc[sel]
            order = np.argsort(ed, kind="stable")
            ed = ed[order]
            es = es[order]
            starts = np.searchsorted(ed, np.arange(PERCORE))
            rank = np.arange(len(ed)) - starts[ed]
            kmax = int(K[c].max())
            M = np.full((PC_PAD, kmax), CHUNK, np.int16)
            M[invperms[i, c][ed], rank] = es.astype(np.int16)
            Ms[i][c] = M

    # greedy-pack tiles into calls per chunk (same packing all cores)
    calls = []  # list of (chunk, [(tile, K_t, col_off)], total_cols)
    for c in range(NCHUNK):
        cur, cols = [], 0
        for t in range(NTILES):
            kt = int(K[c][t])
            if cur and cols + kt > MAX_CALL_COLS:
                calls.append((c, cur, cols))
                cur, cols = [], 0
            cur.append((t, kt, cols))
            cols += kt
        if cur:
            calls.append((c, cur, cols))

    # build gather idx tensor per core: concat over calls of wrapped positions
    idxg = []
    for i in range(NC):
        parts = []
        for (c, tiles, tot) in calls:
            vals = np.empty(tot * 128, np.int16)
            for (t, kt, off) in tiles:
                blk = Ms[i][c][t * 128:(t + 1) * 128, :kt]   # [128, kt]
                # position (col*128 + p) -> (p, col): c-major = blk.T.ravel()
                vals[off * 128:(off + kt) * 128] = blk.T.ravel()
            parts.append(_wrap_idx(vals))
        idxg.append(np.concatenate(parts, axis=1))
    idxg = np.stack(idxg)  # [NC, 128, COLS_G]

    # combine idx: per tile (natural order), call A: chunks 0/1 rows in acc01,
    # call B: chunks 2/3 rows in acc23. K=2 each.
    idxc = []
    for i in range(NC):
        parts = []
        for t in range(NTILES):
            d = np.arange(t * 128, (t + 1) * 128)
            a0 = invperms[i, 0][d]
            a1 = PC_PAD + invperms[i, 1][d]
            a2 = invperms[i, 2][d]
            a3 = PC_PAD + invperms[i, 3][d]
            va = np.concatenate([a0, a1]).astype(np.int16)   # positions col0,col1
            vb = np.concatenate([a2, a3]).astype(np.int16)
            parts.append(_wrap_idx(va))
            parts.append(_wrap_idx(vb))
        idxc.append(np.concatenate(parts, axis=1))
    idxc = np.stack(idxc)  # [NC, 128, COLS_C]

    return dis, perms, calls, K, idxg, idxc


# ---------------- bass kernel builders ----------------

def _bass_mods():
    import sys
    if "/opt/trn_rl_repo" not in sys.path:
        sys.path.insert(0, "/opt/trn_rl_repo")
    import concourse.bass as bass
    import concourse.bacc as bacc
    import concourse.tile as tile
    from concourse import mybir
    from concourse.bass_utils import run_bass_kernel_spmd
    return bass, bacc, tile, mybir, run_bass_kernel_spmd


def _build_mm(NQ=1):
    """h1p = (x @ W1) * dis for own 12544-node shard. xT input pre-transposed."""
    bass, bacc, tile, mybir, _ = _bass_mods()
    from contextlib import ExitStack
    nc = bacc.Bacc()
    xT = nc.declare_dram_parameter("xT", [E_CH, PC_PAD], mybir.dt.float32, isOutput=False)
    W1 = nc.declare_dram_parameter("W1", [E_CH, HID], mybir.dt.float32, isOutput=False)
    disp = nc.declare_dram_parameter("disp", [PC_PAD, 1], mybir.dt.float32, isOutput=False)
    out = nc.declare_dram_parameter("out", [PC_PAD, HID], mybir.dt.float32, isOutput=True)
    with tile.TileContext(nc) as tc, ExitStack() as ctx:
        wp = ctx.enter_context(tc.tile_pool(name="wp", bufs=1))
        sb = ctx.enter_context(tc.tile_pool(name="sb", bufs=4))
        ps = ctx.enter_context(tc.tile_pool(name="ps", bufs=4, space="PSUM"))
        w1 = wp.tile([E_CH, HID], mybir.dt.float32, tag="w1")
        nc.sync.dma_start(out=w1[:], in_=W1[:, :])
        for t in range(NTILES):
            xt = sb.tile([E_CH, 128], mybir.dt.float32, tag="xt")
            nc.sync.dma_start(out=xt[:], in_=xT[:, t * 128:(t + 1) * 128])
            dt_ = sb.tile([128, 1], mybir.dt.float32, tag="dt")
            nc.sync.dma_start(out=dt_[:], in_=disp[t * 128:(t + 1) * 128, :])
            pt = ps.tile([128, HID], mybir.dt.float32, space="PSUM", tag="pt")
            nc.tensor.matmul(pt[:], lhsT=xt[:], rhs=w1[:], start=True, stop=True)
            ot = sb.tile([128, HID], mybir.dt.float32, tag="ot")
            nc.vector.tensor_scalar_mul(ot[:], pt[:], dt_[:])
            nc.sync.dma_start(out=out[t * 128:(t + 1) * 128, :], in_=ot[:])
    nc.compile()
    return nc


def _build_agg(calls, K, cols_g, cols_c, layer):
    """Aggregation launch. layer=1: epilogue relu(dis*(dis*S+b1)) -> [PC_PAD, 64].
    layer=2: epilogue (S*dis)@W2 + b2 -> [PC_PAD, 2]."""
    bass, bacc, tile, mybir, _ = _bass_mods()
    from contextlib import ExitStack
    from concourse.masks import make_identity
    NQ = 4
    nc = bacc.Bacc(num_swdge_queues=NQ, dynamic_dma_scratch_size=16384 * NQ)
    tabs = [nc.declare_dram_parameter(f"tab{c}", [CHUNK_ROWS, HID], mybir.dt.float32,
                                      isOutput=False) for c in range(NCHUNK)]
    idxg = nc.declare_dram_parameter("idxg", [128, cols_g], mybir.dt.int16, isOutput=False)
    idxc = nc.declare_dram_parameter("idxc", [128, cols_c], mybir.dt.int16, isOutput=False)
    disp = nc.declare_dram_parameter("disp", [PC_PAD, 1], mybir.dt.float32, isOutput=False)
    if layer == 1:
        b1b = nc.declare_dram_parameter("b1b", [128, HID], mybir.dt.float32, isOutput=False)
        out = nc.declare_dram_parameter("out", [PC_PAD, HID], mybir.dt.float32, isOutput=True)
    else:
        W2 = nc.declare_dram_parameter("W2", [HID, OUT], mybir.dt.float32, isOutput=False)
        b2b = nc.declare_dram_parameter("b2b", [128, OUT], mybir.dt.float32, isOutput=False)
        out = nc.declare_dram_parameter("out", [PC_PAD, OUT], mybir.dt.float32, isOutput=True)
    acc01 = nc.dram_tensor("acc01", [2 * PC_PAD, HID], mybir.dt.float32)
    acc23 = nc.dram_tensor("acc23", [2 * PC_PAD, HID], mybir.dt.float32)
    accs = [acc01, acc01, acc23, acc23]
    accoff = [0, PC_PAD, 0, PC_PAD]

    with tile.TileContext(nc) as tc, ExitStack() as ctx:
        cst = ctx.enter_context(tc.tile_pool(name="cst", bufs=1))
        ib = ctx.enter_context(tc.tile_pool(name="ib", bufs=2))
        stp = ctx.enter_context(tc.tile_pool(name="stp", bufs=3))
        ab = ctx.enter_context(tc.tile_pool(name="ab", bufs=4))
        ep = ctx.enter_context(tc.tile_pool(name="ep", bufs=4))
        ps = ctx.enter_context(tc.tile_pool(name="ps", bufs=4, space="PSUM"))

        if layer == 1:
            b1t = cst.tile([128, HID], mybir.dt.float32, tag="b1t")
            nc.sync.dma_start(out=b1t[:], in_=b1b[:, :])
        else:
            w2t = cst.tile([HID, OUT], mybir.dt.float32, tag="w2t")
            nc.sync.dma_start(out=w2t[:], in_=W2[:, :])
            b2t = cst.tile([128, OUT], mybir.dt.float32, tag="b2t")
            nc.sync.dma_start(out=b2t[:], in_=b2b[:, :])
            ident = cst.tile([128, 128], mybir.dt.float32, tag="ident")
            make_identity(nc, ident[:])

        # ---- chunk passes ----
        qn = 0
        goff = 0  # column offset into idxg (int16 cols = positions/16)
        # per-chunk idx SBUF tiles loaded lazily per pass
        cur_chunk = -1
        idx_sb = None
        chunk_goff = 0
        # precompute per-chunk column extents
        chunk_cols = {c: sum(tot for (cc, _, tot) in calls if cc == c) for c in range(NCHUNK)}
        for (c, tiles, tot) in calls:
            if c != cur_chunk:
                cur_chunk = c
                chunk_goff = goff
                ccols = chunk_cols[c] * 8  # int16 cols per pass (= positions/16)
                idx_sb = ib.tile([128, ccols], mybir.dt.int16, tag="idx")
                nc.gpsimd.dma_start(out=idx_sb[:], in_=idxg[:, goff:goff + ccols])
            ni = tot * 128
            stage = stp.tile([128, tot * HID], mybir.dt.float32, tag="stage")
            lo = (goff - chunk_goff)
            nc.gpsimd.dma_gather(
                out_ap=stage[:].rearrange("p (k f) -> p k f", k=tot),
                in_ap=tabs[c][:, :],
                idxs_ap=idx_sb[:, lo:lo + tot * 8],
                num_idxs=ni,
                num_idxs_reg=ni,
                elem_size=HID,
                queue_num=qn,
                single_packet=False,
            )
            qn = (qn + 1) % NQ
            for (t, kt, off) in tiles:
                at = ab.tile([128, HID], mybir.dt.float32, tag="at")
                nc.vector.tensor_reduce(
                    out=at[:],
                    in_=bass.AP(stage.tensor, stage[:].offset + off * HID,
                                [stage[:].ap[0], [1, HID], [HID, kt]]),
                    axis=mybir.AxisListType.X,
                    op=mybir.AluOpType.add,
                )
                nc.sync.dma_start(
                    out=accs[c][accoff[c] + t * 128:accoff[c] + (t + 1) * 128, :],
                    in_=at[:])
            goff += tot * 8

        # ---- combine + epilogue ----
        coff = 0
        for t in range(NTILES):
            stage = stp.tile([128, 4 * HID], mybir.dt.float32, tag="cstage")
            icA = ib.tile([128, 16], mybir.dt.int16, tag="icA")
            nc.gpsimd.dma_start(out=icA[:], in_=idxc[:, coff:coff + 16])
            nc.gpsimd.dma_gather(
                out_ap=stage[:, :2 * HID].rearrange("p (k f) -> p k f", k=2),
                in_ap=acc01[:, :],
                idxs_ap=icA[:, :],
                num_idxs=256, num_idxs_reg=256, elem_size=HID,
                queue_num=qn, single_packet=False)
            qn = (qn + 1) % NQ
            icB = ib.tile([128, 16], mybir.dt.int16, tag="icB")
            nc.gpsimd.dma_start(out=icB[:], in_=idxc[:, coff + 16:coff + 32])
            nc.gpsimd.dma_gather(
                out_ap=stage[:, 2 * HID:].rearrange("p (k f) -> p k f", k=2),
                in_ap=acc23[:, :],
                idxs_ap=icB[:, :],
                num_idxs=256, num_idxs_reg=256, elem_size=HID,
                queue_num=qn, single_packet=False)
            qn = (qn + 1) % NQ
            coff += 32
            s = ep.tile([128, HID], mybir.dt.float32, tag="s")
            nc.vector.tensor_reduce(
                out=s[:],
                in_=bass.AP(stage.tensor, stage[:].offset,
                            [stage[:].ap[0], [1, HID], [HID, 4]]),
                axis=mybir.AxisListType.X,
                op=mybir.AluOpType.add,
            )
            dt_ = ep.tile([128, 1], mybir.dt.float32, tag="dt")
            nc.sync.dma_start(out=dt_[:], in_=disp[t * 128:(t + 1) * 128, :])
            u = ep.tile([128, HID], mybir.dt.float32, tag="u")
            nc.vector.tensor_scalar_mul(u[:], s[:], dt_[:])
            if layer == 1:
                v = ep.tile([128, HID], mybir.dt.float32, tag="v")
                nc.vector.tensor_add(v[:], u[:], b1t[:])
                w = ep.tile([128, HID], mybir.dt.float32, tag="w")
                nc.vector.tensor_scalar_mul(w[:], v[:], dt_[:])
                r = ep.tile([128, HID], mybir.dt.float32, tag="r")
                nc.vector.tensor_scalar_max(r[:], w[:], 0.0)
                nc.sync.dma_start(out=out[t * 128:(t + 1) * 128, :], in_=r[:])
            else:
                # (u) @ W2 + b2 : transpose u via PE, matmul
                put = ps.tile([HID, 128], mybir.dt.float32, space="PSUM", tag="put")
                nc.tensor.transpose(out=put[:], in_=u[:], identity=ident[:])
                ut = ep.tile([HID, 128], mybir.dt.float32, tag="ut")
                nc.vector.tensor_copy(out=ut[:], in_=put[:])
                po = ps.tile([128, OUT], mybir.dt.float32, space="PSUM", tag="po")
                nc.tensor.matmul(po[:], lhsT=ut[:], rhs=w2t[:], start=True, stop=True)
                o = ep.tile([128, OUT], mybir.dt.float32, tag="o")
                nc.vector.tensor_add(o[:], po[:], b2t[:])
                nc.sync.dma_start(out=out[t * 128:(t + 1) * 128, :], in_=o[:])
    nc.compile()
    return nc


def _mk_tables(h):
    """h [N, 64] f32 -> 4 chunk tensors [25001, 64] with zero row at local 25000."""
    tabs = []
    for c in range(NCHUNK):
        t = np.zeros((CHUNK_ROWS, HID), np.float32)
        t[:CHUNK] = h[c * CHUNK:(c + 1) * CHUNK]
        tabs.append(t)
    return tabs


def kernel(x, edge_index, W1, b1, W2, b2):
    x = np.asarray(x, dtype=np.float32)
    W1 = np.asarray(W1, dtype=np.float32)
    b1 = np.asarray(b1, dtype=np.float32)
    W2 = np.asarray(W2, dtype=np.float32)
    b2 = np.asarray(b2, dtype=np.float32)

    bass, bacc, tile, mybir, run_spmd = _bass_mods()

    dis, perms, calls, K, idxg, idxc = _host_prep(edge_index)
    cols_g = idxg.shape[2]
    cols_c = idxc.shape[2]
    dis_pad = np.concatenate([dis, np.ones(PC_PAD - PERCORE, np.float32)])

    cores = list(range(NC))

    # ---- launch 1: h1p = (x @ W1) * dis ----
    nc1 = _build_mm()
    in1 = []
    for i in cores:
        xT = np.zeros((E_CH, PC_PAD), np.float32)
        xT[:, :PERCORE] = x[i * PERCORE:(i + 1) * PERCORE].T
        dp = dis_pad.copy()
        dp[:PERCORE] = dis[i * PERCORE:(i + 1) * PERCORE]
        in1.append({"xT": xT, "W1": W1, "disp": dp[:, None]})
    r1 = run_spmd(nc1, in1, core_ids=cores)
    h1p = np.concatenate([np.asarray(r1.results[i]["out"])[:PERCORE] for i in cores])

    # ---- launch 2: layer-1 aggregation + activation -> a1p ----
    nc2 = _build_agg(calls, K, cols_g, cols_c, layer=1)
    tabs1 = _mk_tables(h1p)
    b1bc = np.broadcast_to(b1, (128, HID)).copy()
    in2 = []
    for i in cores:
        dp = dis_pad.copy()
        dp[:PERCORE] = dis[i * PERCORE:(i + 1) * PERCORE]
        m = {f"tab{c}": tabs1[c] for c in range(NCHUNK)}
        m.update({"idxg": idxg[i], "idxc": idxc[i], "disp": dp[:, None], "b1b": b1bc})
        in2.append(m)
    r2 = run_spmd(nc2, in2, core_ids=cores)
    a1p = np.concatenate([np.asarray(r2.results[i]["out"])[:PERCORE] for i in cores])

    # ---- launch 3: layer-2 aggregation + head -> out ----
    nc3 = _build_agg(calls, K, cols_g, cols_c, layer=2)
    tabs2 = _mk_tables(a1p)
    b2bc = np.broadcast_to(b2, (128, OUT)).copy()
    in3 = []
    for i in cores:
        dp = dis_pad.copy()
        dp[:PERCORE] = dis[i * PERCORE:(i + 1) * PERCORE]
        m = {f"tab{c}": tabs2[c] for c in range(NCHUNK)}
        m.update({"idxg": idxg[i], "idxc": idxc[i], "disp": dp[:, None],
                  "W2": W2, "b2b": b2bc})
        in3.append(m)
    r3 = run_spmd(nc3, in3, core_ids=cores)
    outv = np.concatenate([np.asarray(r3.results[i]["out"])[:PERCORE] for i in cores])
    return outv.astype(np.float32)
